# revision 1
# baseline (speedup 1.0000x reference)
"""Trainium2 Bass kernel for the MHA-with-diagonal-softmax module.

Computation (per batch b):
    q = rope(x @ Wq.T), k = rope(x @ Wk.T), v = x @ Wv.T      (per head, DH=128)
    sumexp[s,h] = sum_k exp(q_h[s] . k_h[k] * DH^-0.5)
    diag[s,h]   = q_h[s] . k_h[s] * DH^-0.5
    w = exp(diag) / sumexp
    out = (w * v) @ Wo.T

Sharding: 8 cores = 2 (batch) x 4 (head groups of 4 heads).
Each core computes q/k/v for its 4 heads in transposed [head_dim, seq]
layout, the per-position softmax-diagonal weights, and a partial output
projection (its heads' rows of Wo), written as 2 head-pair partials that
the host sums.

On-chip dtype is fp16 (same PE throughput as bf16, 8x lower rounding
error - matters because exp() amplifies absolute score error), with fp32
PSUM accumulation everywhere.
"""

import numpy as np
from contextlib import ExitStack

# Problem constants (hardcoded per harness contract).
B, S, D, H, DH = 2, 2048, 2048, 16, 128
HPC = 4            # heads per core
NHL = HPC * DH     # 512 local head dims per core
KB = D // 128      # 16 contraction blocks
SB = S // 128      # 16 seq blocks of 128
SC = S // 512      # 4 seq/emb chunks of 512
NCORES = 8

_CACHE = {}


def _build_nc():
    import concourse.bass as bass
    import concourse.tile as tile
    from concourse import bacc, mybir
    from concourse.masks import make_identity

    F16 = mybir.dt.float16
    F32 = mybir.dt.float32
    AF = mybir.ActivationFunctionType
    ALU = mybir.AluOpType
    AX = mybir.AxisListType

    # Bacc (not raw Bass): its compile() splits multi-sem waits into
    # event-semaphore instructions - HW allows at most 1 wait per inst.
    nc = bacc.Bacc("TRN2", target_bir_lowering=False, debug=False)

    xT = nc.dram_tensor("xT", [D, S], F16, kind="ExternalInput").ap()
    wq = nc.dram_tensor("wq", [D, NHL], F16, kind="ExternalInput").ap()
    wk = nc.dram_tensor("wk", [D, NHL], F16, kind="ExternalInput").ap()
    wv = nc.dram_tensor("wv", [D, NHL], F16, kind="ExternalInput").ap()
    wo = nc.dram_tensor("wo", [NHL, D], F16, kind="ExternalInput").ap()
    ropeA = nc.dram_tensor("ropeA", [128, S], F16, kind="ExternalInput").ap()
    ropeB = nc.dram_tensor("ropeB", [128, S], F16, kind="ExternalInput").ap()
    y = nc.dram_tensor("y", [2, S, D], F16, kind="ExternalOutput").ap()

    xT_r = xT.rearrange("(a p) s -> a p s", p=128)
    wq_r = wq.rearrange("(a p) m -> a p m", p=128)
    wk_r = wk.rearrange("(a p) m -> a p m", p=128)
    wv_r = wv.rearrange("(a p) m -> a p m", p=128)
    wo_r = wo.rearrange("(h p) n -> h p n", p=128)

    with tile.TileContext(nc) as tc, ExitStack() as ctx:
        pool = ctx.enter_context(tc.tile_pool(name="sb", bufs=1))
        pp = ctx.enter_context(tc.tile_pool(name="ps", bufs=1, space="PSUM"))

        # ---- constants ----
        ra = pool.tile([128, S], F16, name="ra")
        rb = pool.tile([128, S], F16, name="rb")
        # SWDGE: a wide HWDGE DMA fans out over several HW queues, and a
        # DVE/ACT consumer then needs one sync-wait per queue, exceeding
        # the instruction's wait-slot budget at compile time.
        nc.gpsimd.dma_start(ra[:, :], ropeA[:, :])
        nc.gpsimd.dma_start(rb[:, :], ropeB[:, :])
        ident = pool.tile([128, 128], F32, name="ident")
        make_identity(nc, ident[:, :])
        onesf = pool.tile([128, 128], F32, name="onesf")
        nc.gpsimd.memset(onesf[:, :], 1.0)
        ones1 = pool.tile([128, 128], F16, name="ones1")
        nc.gpsimd.memset(ones1[:, :], 1.0)

        # ---- x resident in SBUF ----
        xsb = pool.tile([128, KB, S], F16, name="xsb")
        for kb in range(KB):
            nc.sync.dma_start(xsb[:, kb, :], xT_r[kb])

        # ---- persistent q/k/v head tiles ([head_dim, seq] layout) ----
        qh = [pool.tile([128, S], F16, name=f"qh{h}") for h in range(HPC)]
        kh = [pool.tile([128, S], F16, name=f"kh{h}") for h in range(HPC)]
        vh = [pool.tile([128, S], F16, name=f"vh{h}") for h in range(HPC)]

        # per-head row vectors live at partition 32*h (engine ops only
        # support start partitions that are multiples of 32)
        ds_diag = pool.tile([128, S], F32, name="ds_diag")
        ds_sum = pool.tile([128, S], F16, name="ds_sum")
        w4 = pool.tile([128, S], F16, name="w4")
        sumf = [pool.tile([128, SB], F32, name=f"sumf{h}") for h in range(HPC)]

        def load_w(src_r, nblk, tag="w"):
            t = pool.tile([128, nblk, 512 * (KB // nblk)], F16, name="wt",
                          tag=tag, bufs=2)
            for i in range(nblk):
                nc.sync.dma_start(t[:, i, :], src_r[i])
            return t

        def proj_chunk(wt, dests, mt, sc):
            # dests[mt][:, sc-chunk] <- (wt[:, :, mt] block).T @ x chunk
            ps = pp.tile([128, 512], F32, name="mmps", tag="mm", bufs=2)
            for kb in range(KB):
                nc.tensor.matmul(
                    ps[:, :],
                    wt[:, kb, mt * 128:(mt + 1) * 128],
                    xsb[:, kb, sc * 512:(sc + 1) * 512],
                    start=(kb == 0), stop=(kb == KB - 1))
            nc.scalar.activation(
                dests[mt][:, sc * 512:(sc + 1) * 512], ps[:, :], AF.Copy)

        def proj(wt, dests):
            for mt in range(HPC):
                for sc in range(SC):
                    proj_chunk(wt, dests, mt, sc)

        def rope(dst):
            # dst (in place): top = te*cos - to*sin ; bottom = te*sin + to*cos
            # ra = [cosT; cosT], rb = [-sinT; sinT]; swap = halves exchanged.
            for c in range(2):
                sl = slice(c * 1024, (c + 1) * 1024)
                # SWDGE (gpsimd) keeps this 1 queue -> 1 sem; a wide HWDGE
                # sbuf->sbuf DMA fans out over many queues and blows the
                # consumer's sync-wait slot budget.
                swp = pool.tile([128, 1024], F16, name="swp", tag="swp", bufs=1)
                nc.gpsimd.dma_start(swp[0:64, :], dst[64:128, sl])
                nc.gpsimd.dma_start(swp[64:128, :], dst[0:64, sl])
                u = pool.tile([128, 1024], F16, name="u", tag="sc", bufs=2)
                nc.vector.tensor_mul(u[:, :], dst[:, sl], ra[:, sl])
                v2 = pool.tile([128, 1024], F16, name="v2", tag="sc", bufs=2)
                nc.vector.tensor_mul(v2[:, :], swp[:, :], rb[:, sl])
                nc.vector.tensor_add(dst[:, sl], u[:, :], v2[:, :])

        def diag(h):
            # ds_diag[32h, s] = sum_m qh[h][m, s] * kh[h][m, s]  (fp32)
            hp = 32 * h
            for c in range(2):
                sl = slice(c * 1024, (c + 1) * 1024)
                pr = pool.tile([128, 1024], F32, name="pr", tag="pr", bufs=1)
                nc.vector.tensor_mul(pr[:, :], qh[h][:, sl], kh[h][:, sl])
                for cc in range(2):
                    dps = pp.tile([128, 512], F32, name="dps", tag="sm", bufs=2)
                    nc.tensor.matmul(dps[:, :], onesf[:, :],
                                     pr[:, cc * 512:(cc + 1) * 512],
                                     start=True, stop=True)
                    o = (2 * c + cc) * 512
                    nc.scalar.activation(ds_diag[hp:hp + 1, o:o + 512],
                                         dps[hp:hp + 1, :], AF.Copy)

        # ====== phase 2 pieces ======
        def scores_sq(h, sq, fillers):
            """One query block: 4 score MMs into a 4-bank psum tile, one wide
            exp with fused row-sum. `fillers` = list of callables emitting
            independent PE work, interleaved so the in-order PE queue always
            has runnable matmuls while ACT drains the exp (keeps HAM warm)."""
            sps = pp.tile([128, S], F32, name="sps", tag="sco", bufs=1)
            for ck in range(SC):
                nc.tensor.matmul(sps[:, ck * 512:(ck + 1) * 512],
                                 qh[h][:, sq * 128:(sq + 1) * 128],
                                 kh[h][:, ck * 512:(ck + 1) * 512],
                                 start=True, stop=True)
            for f in fillers:
                f()
            ex = pool.tile([128, S], F16, name="ex", tag="ex", bufs=1)
            nc.scalar.activation(ex[:, :], sps[:, :], AF.Exp,
                                 accum_out=sumf[h][:, sq:sq + 1])

        def head_sum_tail(h):
            # recip -> transpose -> [1, S] row of ds_sum
            rs = pool.tile([128, SB], F32, name="rs", tag="rs", bufs=2)
            nc.vector.reciprocal(rs[:, :], sumf[h][:, :])
            tps = pp.tile([16, 128], F32, name="tps", tag="sm", bufs=2)
            nc.tensor.transpose(tps[:, :], rs[:, :], ident[:, :])
            st = pool.tile([16, 128], F16, name="st", tag="st", bufs=2)
            nc.vector.tensor_copy(st[:, :], tps[:, :])
            nc.gpsimd.dma_start(ds_sum[32 * h:32 * h + 1, :], st[:, :])

        def pair_head(p):
            # w = exp(diag) * recip(sumexp); attn = w (bcast) * v, into kh
            h0, h1 = 2 * p, 2 * p + 1
            expd = pool.tile([128, S], F16, name="expd", tag="expd", bufs=2)
            for h in (h0, h1):
                hp = 32 * h
                nc.scalar.activation(expd[hp:hp + 1, :], ds_diag[hp:hp + 1, :],
                                     AF.Exp)
                nc.vector.tensor_mul(w4[hp:hp + 1, :], expd[hp:hp + 1, :],
                                     ds_sum[hp:hp + 1, :])
                wb = pool.tile([128, S], F16, name="wb", tag="wb", bufs=1)
                for ck in range(SC):
                    bps = pp.tile([128, 512], F32, name="bps", tag="mm", bufs=2)
                    nc.tensor.matmul(bps[:, :], ones1[hp:hp + 1, :],
                                     w4[hp:hp + 1, ck * 512:(ck + 1) * 512],
                                     start=True, stop=True,
                                     tile_position=(hp, 0))
                    nc.vector.tensor_copy(wb[:, ck * 512:(ck + 1) * 512],
                                          bps[:, :])
                nc.vector.tensor_mul(kh[h][:, :], wb[:, :], vh[h][:, :])

        def oproj_group(p, sb, ncx):
            h0, h1 = 2 * p, 2 * p + 1
            ps = pp.tile([128, 512], F32, name="ops", tag="mm", bufs=2)
            for i, h in enumerate((h0, h1)):
                nc.tensor.matmul(
                    ps[:, :], kh[h][:, sb * 128:(sb + 1) * 128],
                    wot[:, h, ncx * 512:(ncx + 1) * 512],
                    start=(i == 0), stop=(i == 1))
            yt = pool.tile([128, 512], F16, name="yt", tag="yt", bufs=2)
            nc.vector.tensor_copy(yt[:, :], ps[:, :])
            nc.sync.dma_start(
                y[p, sb * 128:(sb + 1) * 128,
                  ncx * 512:(ncx + 1) * 512], yt[:, :])

        # ================= emission =================
        # dense PE phase: K and Q projections + rope + diag
        wkt = load_w(wk_r, KB)
        wqt = load_w(wq_r, KB)
        proj(wkt, kh)
        for h in range(HPC):
            rope(kh[h])
        proj(wqt, qh)
        for h in range(HPC):
            rope(qh[h])
            diag(h)
        # wv reuses wk's slot, wo reuses wq's slot (tag bufs=2)
        wvt = load_w(wv_r, KB)
        wot = load_w(wo_r, HPC)

        # scores streams, with independent matmul work as filler:
        #   head 0/1 slots <- V projection chunks (16 groups of 16 MMs)
        #   head 2 slots   <- pair-0 output projection (64 groups of 2 MMs)
        #   head 3 slots   <- none available (paced by exp)
        vfill = [(mt, sc) for mt in range(HPC) for sc in range(SC)]
        for sq in range(SB):
            f = []
            if sq % 2 == 0 and vfill:
                mt, sc = vfill.pop(0)
                f.append(lambda mt=mt, sc=sc: proj_chunk(wvt, vh, mt, sc))
            scores_sq(0, sq, f)
        head_sum_tail(0)
        for sq in range(SB):
            f = []
            if sq % 2 == 0 and vfill:
                mt, sc = vfill.pop(0)
                f.append(lambda mt=mt, sc=sc: proj_chunk(wvt, vh, mt, sc))
            scores_sq(1, sq, f)
        head_sum_tail(1)
        pair_head(0)
        ofill = [(sb, ncx) for sb in range(SB) for ncx in range(SC)]
        for sq in range(SB):
            f = []
            for _ in range(4):
                if ofill:
                    sb, ncx = ofill.pop(0)
                    f.append(lambda sb=sb, ncx=ncx: oproj_group(0, sb, ncx))
            scores_sq(2, sq, f)
        head_sum_tail(2)
        for sq in range(SB):
            f = []
            if ofill:
                sb, ncx = ofill.pop(0)
                f.append(lambda sb=sb, ncx=ncx: oproj_group(0, sb, ncx))
            scores_sq(3, sq, f)
        head_sum_tail(3)
        for sb, ncx in ofill:
            oproj_group(0, sb, ncx)
        pair_head(1)
        for sb in range(SB):
            for ncx in range(SC):
                oproj_group(1, sb, ncx)

    nc.compile()
    return nc


def _get_nc():
    if "nc" not in _CACHE:
        _CACHE["nc"] = _build_nc()
    return _CACHE["nc"]


_PERM = np.concatenate([np.arange(0, DH, 2), np.arange(1, DH, 2)])


def _host_inputs(x, rope_cos, rope_sin, Wq, Wk, Wv, Wo):
    """Build the 8 per-core input maps."""
    f16 = np.float16
    cosT = np.ascontiguousarray(np.asarray(rope_cos, np.float32)[0, :, 0, :].T)
    sinT = np.ascontiguousarray(np.asarray(rope_sin, np.float32)[0, :, 0, :].T)
    ra = np.concatenate([cosT, cosT], 0).astype(f16)
    rb = np.concatenate([-sinT, sinT], 0).astype(f16)

    Wq = np.asarray(Wq, np.float32)
    Wk = np.asarray(Wk, np.float32)
    Wv = np.asarray(Wv, np.float32)
    Wo = np.asarray(Wo, np.float32)
    x = np.asarray(x, np.float32)

    xTb = [np.ascontiguousarray(x[b].T).astype(f16) for b in range(B)]
    scale = DH ** -0.5

    in_maps = []
    for core in range(NCORES):
        b, g = divmod(core, HPC)
        hs = g * HPC
        rows = np.concatenate(
            [h * DH + _PERM for h in range(hs, hs + HPC)])      # deinterleave
        rows_v = np.arange(hs * DH, (hs + HPC) * DH)
        in_maps.append({
            "xT": xTb[b],
            "wq": np.ascontiguousarray((Wq[rows] * scale).T).astype(f16),
            "wk": np.ascontiguousarray(Wk[rows].T).astype(f16),
            "wv": np.ascontiguousarray(Wv[rows_v].T).astype(f16),
            "wo": np.ascontiguousarray(Wo[:, rows_v].T).astype(f16),
            "ropeA": ra,
            "ropeB": rb,
        })
    return in_maps


def kernel(x, rope_cos, rope_sin, Wq, Wk, Wv, Wo, _trace=False, _trace_cores=None):
    from concourse.bass_utils import run_bass_kernel_spmd

    nc = _get_nc()
    in_maps = _host_inputs(x, rope_cos, rope_sin, Wq, Wk, Wv, Wo)
    res = run_bass_kernel_spmd(nc, in_maps, list(range(NCORES)),
                               trace=_trace, trace_cores=_trace_cores)
    _CACHE["last_result"] = res

    out = np.zeros((B, S, D), np.float32)
    for core in range(NCORES):
        b = core // HPC
        out[b] += res.results[core]["y"].astype(np.float32).sum(axis=0)
    return out



# revision 10
# speedup vs baseline: 1.3129x; 1.3129x over previous
"""Trainium2 Bass kernel for the MHA-with-diagonal-softmax module.

Computation (per batch b):
    q = rope(x @ Wq.T), k = rope(x @ Wk.T), v = x @ Wv.T      (per head, DH=128)
    sumexp[s,h] = sum_k exp(q_h[s] . k_h[k] * DH^-0.5)
    w = exp(q_h[s] . k_h[s] * DH^-0.5) / sumexp
    out = (w * v) @ Wo.T

Sharding: 8 cores = 2 (batch) x 4 (head groups of 4 heads).
Each core computes q/k/v for its 4 heads in transposed [head_dim, seq]
layout, then streams phase 2 seq-block-major: for each 128-query block,
scores+exp for all 4 heads, attention weights (the diagonal exp is
extracted from the exp output with an identity mask), w*v, and the
4-head-accumulated output projection - so y DMA flows continuously
instead of serializing at the end.  Host sums the 4 per-core partials
per batch.

On-chip dtype is fp16 with fp32 PSUM accumulation.
"""

import numpy as np
from contextlib import ExitStack

# Problem constants (hardcoded per harness contract).
B, S, D, H, DH = 2, 2048, 2048, 16, 128
HPC = 4            # heads per core
NHL = HPC * DH     # 512 local head dims per core
KB = D // 128      # 16 contraction blocks
SB = S // 128      # 16 seq blocks of 128
SC = S // 512      # 4 seq/emb chunks of 512
NCORES = 8

_CACHE = {}


def _build_nc():
    import concourse.bass as bass
    import concourse.tile as tile
    from concourse import bacc, mybir
    from concourse.masks import make_identity

    F16 = mybir.dt.float16
    F32 = mybir.dt.float32
    AF = mybir.ActivationFunctionType
    ALU = mybir.AluOpType
    AX = mybir.AxisListType

    nc = bacc.Bacc("TRN2", target_bir_lowering=False, debug=False)

    xT = nc.dram_tensor("xT", [D, S], F16, kind="ExternalInput").ap()
    wq = nc.dram_tensor("wq", [D, NHL], F16, kind="ExternalInput").ap()
    wk = nc.dram_tensor("wk", [D, NHL], F16, kind="ExternalInput").ap()
    wv = nc.dram_tensor("wv", [D, NHL], F16, kind="ExternalInput").ap()
    wo = nc.dram_tensor("wo", [NHL, D], F16, kind="ExternalInput").ap()
    ropeA = nc.dram_tensor("ropeA", [128, S], F16, kind="ExternalInput").ap()
    ropeB = nc.dram_tensor("ropeB", [128, S], F16, kind="ExternalInput").ap()
    y = nc.dram_tensor("y", [S, D], F16, kind="ExternalOutput").ap()

    xT_r = xT.rearrange("(a p) s -> a p s", p=128)
    wq_r = wq.rearrange("(a p) m -> a p m", p=128)
    wk_r = wk.rearrange("(a p) m -> a p m", p=128)
    wv_r = wv.rearrange("(a p) m -> a p m", p=128)
    wo_r = wo.rearrange("(h p) n -> h p n", p=128)

    with tile.TileContext(nc) as tc, ExitStack() as ctx:
        pool = ctx.enter_context(tc.tile_pool(name="sb", bufs=1))
        pp = ctx.enter_context(tc.tile_pool(name="ps", bufs=1, space="PSUM"))

        # ---- constants ----
        ra = pool.tile([128, S], F16, name="ra")
        rb = pool.tile([128, S], F16, name="rb")
        # SWDGE: wide HWDGE DMA fans out over several HW queues and the
        # DVE consumer would need one sync-wait per queue (over budget).
        nc.gpsimd.dma_start(ra[:, :], ropeA[:, :])
        nc.gpsimd.dma_start(rb[:, :], ropeB[:, :])
        identf = pool.tile([128, 128], F16, name="identf")
        make_identity(nc, identf[:, :])
        ident32 = pool.tile([128, 128], F32, name="ident32")
        make_identity(nc, ident32[:, :])
        ones1 = pool.tile([128, 128], F16, name="ones1")
        nc.gpsimd.memset(ones1[:, :], 1.0)

        # ---- input DMAs (emission order == queue service order) ----
        xsb = pool.tile([128, KB, S], F16, name="xsb")

        def load_w(src_r, nblk, tag="w"):
            t = pool.tile([128, nblk, 512 * (KB // nblk)], F16, name="wt",
                          tag=tag, bufs=2)
            for i in range(nblk):
                nc.sync.dma_start(t[:, i, :], src_r[i])
            return t

        wkt = load_w(wk_r, KB)                       # K weights first
        for kb in range(KB):                         # x chunk sc=0
            nc.sync.dma_start(xsb[:, kb, 0:512], xT_r[kb][:, 0:512])
        wqt = load_w(wq_r, KB, tag="w2")
        for sc in range(1, SC):                      # rest of x
            for kb in range(KB):
                nc.sync.dma_start(xsb[:, kb, sc * 512:(sc + 1) * 512],
                                  xT_r[kb][:, sc * 512:(sc + 1) * 512])

        # ---- persistent per-head tiles ([head_dim, seq] layout) ----
        qh = [pool.tile([128, S], F16, name=f"qh{h}") for h in range(HPC)]
        kh = [pool.tile([128, S], F16, name=f"kh{h}") for h in range(HPC)]
        vh = [pool.tile([128, S], F16, name=f"vh{h}") for h in range(HPC)]
        ah = vh  # attn mul writes vh in place (each v block consumed once)

        sumf = pool.tile([128, SB, HPC], F32, name="sumf")
        edf = pool.tile([128, SB, HPC], F32, name="edf")

        def proj_chunk(wt, dests, mt, sc):
            # dests[mt][:, sc-chunk] <- (wt[:, :, mt] block).T @ x chunk
            ps = pp.tile([128, 512], F32, name="mmps", tag="mm", bufs=2)
            for kb in range(KB):
                nc.tensor.matmul(
                    ps[:, :],
                    wt[:, kb, mt * 128:(mt + 1) * 128],
                    xsb[:, kb, sc * 512:(sc + 1) * 512],
                    start=(kb == 0), stop=(kb == KB - 1))
            dst = dests[mt][:, sc * 512:(sc + 1) * 512]
            nc.scalar.activation(dst, ps[:, :], AF.Copy)

        def rope(dst):
            # dst (in place): top = te*cos - to*sin ; bottom = te*sin + to*cos
            # ra = [cosT; cosT], rb = [-sinT; sinT]; swap = halves exchanged.
            for c in range(2):
                sl = slice(c * 1024, (c + 1) * 1024)
                swp = pool.tile([128, 1024], F16, name="swp", tag="swp", bufs=1)
                nc.gpsimd.dma_start(swp[0:64, :], dst[64:128, sl])
                nc.gpsimd.dma_start(swp[64:128, :], dst[0:64, sl])
                u = pool.tile([128, 1024], F16, name="u", tag="sc", bufs=2)
                nc.vector.tensor_mul(u[:, :], dst[:, sl], ra[:, sl])
                v2 = pool.tile([128, 1024], F16, name="v2", tag="sc", bufs=2)
                nc.vector.tensor_mul(v2[:, :], swp[:, :], rb[:, sl])
                nc.vector.tensor_add(dst[:, sl], u[:, :], v2[:, :])

        # ================= phase B: K and Q projections + rope =========
        for sc in range(SC):
            for mt in range(HPC):
                proj_chunk(wkt, kh, mt, sc)
                if sc == SC - 1:
                    rope(kh[mt])
        for sc in range(SC):
            for mt in range(HPC):
                proj_chunk(wqt, qh, mt, sc)
                if sc == SC - 1:
                    rope(qh[mt])

        # wv reuses wk's slot, wo reuses wq's slot (tag bufs=2)
        wvt = load_w(wv_r, KB)
        wot = load_w(wo_r, HPC, tag="w2")

        # V chunks for the first seq quarter must precede the streaming loop
        # (attn of blocks 0-3 needs them immediately).
        for h in range(HPC):
            proj_chunk(wvt, vh, h, 0)

        # ================= phase C: streaming scores/attn/oproj ========
        # Remaining V chunks, scheduled 1/iteration as PE filler.
        vfill = [(h, sc) for sc in range(1, SC) for h in range(HPC)]

        def scores_head(h, sq, ex_t):
            # scores for 128 queries x all keys -> exp into ex_t, two halves
            # (psum double-buffered so the next head's matmuls overlap ACT).
            for half in range(2):
                sps = pp.tile([128, 1024], F32, name="sps", tag="sco", bufs=2)
                for c in range(2):
                    o = half * 1024 + c * 512
                    nc.tensor.matmul(sps[:, c * 512:(c + 1) * 512],
                                     qh[h][:, sq * 128:(sq + 1) * 128],
                                     kh[h][:, o:o + 512],
                                     start=True, stop=True)
                nc.scalar.activation(ex_t[:, half * 1024:(half + 1) * 1024],
                                     sps[:, :], AF.Exp)
            # row sum of exp (DVE; gpsimd can only reduce over partitions)
            nc.vector.tensor_reduce(sumf[:, sq, h:h + 1], ex_t[:, :],
                                    AX.X, ALU.add)
            # diagonal extract: ex block sq has exp(diag) on its diagonal
            # (tensor_tensor_reduce is fatal on this runtime - use 2 ops)
            ej = pool.tile([128, 128], F16, name="ej", tag="ej", bufs=2)
            nc.vector.tensor_mul(ej[:, :], ex_t[:, sq * 128:(sq + 1) * 128],
                                 identf[:, :])
            nc.vector.tensor_reduce(edf[:, sq, h:h + 1], ej[:, :],
                                    AX.X, ALU.add)

        def w_chain(sq):
            # w[q] = exp(diag)/sumexp per head -> [1,512] row -> broadcast
            rec = pool.tile([128, HPC], F32, name="rec", tag="rec", bufs=2)
            nc.vector.reciprocal(rec[:, :], sumf[:, sq, :])
            wcol = pool.tile([128, HPC], F32, name="wcol", tag="wcol", bufs=2)
            nc.vector.tensor_mul(wcol[:, :], edf[:, sq, :], rec[:, :])
            tps = pp.tile([HPC, 128], F32, name="tps", tag="tps", bufs=1)
            nc.tensor.transpose(tps[:, :], wcol[:, :], ident32[:, :])
            w4 = pool.tile([HPC, 128], F16, name="w4", tag="w4", bufs=2)
            nc.vector.tensor_copy(w4[:, :], tps[:, :])
            wrow = pool.tile([1, NHL], F16, name="wrow", tag="wrow", bufs=2)
            nc.gpsimd.dma_start(wrow[0:1, :], w4[:, :])
            bps = pp.tile([128, NHL], F32, name="bps", tag="bps", bufs=1)
            nc.tensor.matmul(bps[:, :], ones1[0:1, :], wrow[0:1, :],
                             start=True, stop=True)
            for h in range(HPC):
                nc.vector.tensor_mul(ah[h][:, sq * 128:(sq + 1) * 128],
                                     bps[:, h * 128:(h + 1) * 128],
                                     vh[h][:, sq * 128:(sq + 1) * 128])

        def oproj_piece(sq, ncx):
            # y[sq block, ncx chunk] = sum_h ah[h].T @ wo rows (4-head accum)
            ps = pp.tile([128, 512], F32, name="ops", tag="mm", bufs=2)
            for h in range(HPC):
                nc.tensor.matmul(
                    ps[:, :], ah[h][:, sq * 128:(sq + 1) * 128],
                    wot[:, h, ncx * 512:(ncx + 1) * 512],
                    start=(h == 0), stop=(h == HPC - 1))
            yt = pool.tile([128, 512], F16, name="yt", tag="yt", bufs=2)
            nc.vector.tensor_copy(yt[:, :], ps[:, :])
            nc.sync.dma_start(
                y[sq * 128:(sq + 1) * 128, ncx * 512:(ncx + 1) * 512],
                yt[:, :])

        for sq in range(SB):
            # PE fillers this iteration: oproj of previous block + 1 V chunk
            fills = [lambda n=ncx: oproj_piece(sq - 1, n) for ncx in range(4)] \
                if sq > 0 else []
            if vfill:
                h, sc = vfill.pop(0)
                fills.append(lambda h=h, sc=sc: proj_chunk(wvt, vh, h, sc))
            for h in range(HPC):
                ex_t = pool.tile([128, S], F16, name="ex", tag="ex", bufs=2)
                scores_head(h, sq, ex_t)
                if fills:
                    fills.pop(0)()
            for f in fills:
                f()
            w_chain(sq)
        for ncx in range(4):
            oproj_piece(SB - 1, ncx)

    nc.compile()
    return nc


def _get_nc():
    if "nc" not in _CACHE:
        _CACHE["nc"] = _build_nc()
    return _CACHE["nc"]


_PERM = np.concatenate([np.arange(0, DH, 2), np.arange(1, DH, 2)])


def _host_inputs(x, rope_cos, rope_sin, Wq, Wk, Wv, Wo):
    """Build the 8 per-core input maps."""
    f16 = np.float16
    cosT = np.ascontiguousarray(np.asarray(rope_cos, np.float32)[0, :, 0, :].T)
    sinT = np.ascontiguousarray(np.asarray(rope_sin, np.float32)[0, :, 0, :].T)
    ra = np.concatenate([cosT, cosT], 0).astype(f16)
    rb = np.concatenate([-sinT, sinT], 0).astype(f16)

    Wq = np.asarray(Wq, np.float32)
    Wk = np.asarray(Wk, np.float32)
    Wv = np.asarray(Wv, np.float32)
    Wo = np.asarray(Wo, np.float32)
    x = np.asarray(x, np.float32)

    xTb = [np.ascontiguousarray(x[b].T).astype(f16) for b in range(B)]
    scale = DH ** -0.5

    in_maps = []
    for core in range(NCORES):
        b, g = divmod(core, HPC)
        hs = g * HPC
        rows = np.concatenate(
            [h * DH + _PERM for h in range(hs, hs + HPC)])      # deinterleave
        rows_v = np.arange(hs * DH, (hs + HPC) * DH)
        in_maps.append({
            "xT": xTb[b],
            "wq": np.ascontiguousarray((Wq[rows] * scale).T).astype(f16),
            "wk": np.ascontiguousarray(Wk[rows].T).astype(f16),
            "wv": np.ascontiguousarray(Wv[rows_v].T).astype(f16),
            "wo": np.ascontiguousarray(Wo[:, rows_v].T).astype(f16),
            "ropeA": ra,
            "ropeB": rb,
        })
    return in_maps


def kernel(x, rope_cos, rope_sin, Wq, Wk, Wv, Wo, _trace=False, _trace_cores=None):
    from concourse.bass_utils import run_bass_kernel_spmd

    nc = _get_nc()
    in_maps = _host_inputs(x, rope_cos, rope_sin, Wq, Wk, Wv, Wo)
    res = run_bass_kernel_spmd(nc, in_maps, list(range(NCORES)),
                               trace=_trace, trace_cores=_trace_cores)
    _CACHE["last_result"] = res

    out = np.zeros((B, S, D), np.float32)
    for core in range(NCORES):
        b = core // HPC
        out[b] += res.results[core]["y"].astype(np.float32)
    return out
